# revision 1
# baseline (speedup 1.0000x reference)
"""CronRootAttention (causal sqrt-N sparse attention + GQA projections) on 8 TRN2 cores.

Sharding: pure sequence shard — each core owns 256 queries, computes all 16
heads for them. Weights are replicated; kv projections computed per-core for
the local 384-key span plus the 44 shared strided keys.

Per-core dataflow (all matmuls fp32r = full-speed reduced-precision fp32):
  q_T [1024, 256]  = WqT.T @ xT       (channels on partitions)
  k_T [256, 384], ks_T [256, 44]      (kv channels on partitions)
  v   [384, 260], vs [44, 260]        (rows = keys; 65-stride head layout with
                                       a ones column per head for softmax denom)
  scores_T [j, i] per (kv-head g, j-tile) -> exp (no max-sub; scores are O(5))
  -> mask (host-precomputed 0/1) -> PV matmul accumulates [65, 256] per head:
  rows 0..63 = unnormalized attn_T, row 64 = softmax denominator.
  normalize via reciprocal + DMA partition-broadcast + DVE multiply
  y_T [1024, 256] = WoT.T @ attn_T    -> host concatenates + transposes.
"""

import math
import sys

sys.path.insert(0, "/opt/trn_rl_repo")

import numpy as np
import concourse.bass as bass
import concourse.tile as tile
from concourse import bacc, mybir
from concourse.bass_utils import run_bass_kernel_spmd

F32 = mybir.dt.float32
F32R = mybir.dt.float32r
BF16 = mybir.dt.bfloat16
EXP = mybir.ActivationFunctionType.Exp
import os
import ml_dtypes

USE_BF16 = os.environ.get("BASS_MM_DT", "bf16") == "bf16"
MM_DT = BF16 if USE_BF16 else F32R
NP_DT = ml_dtypes.bfloat16 if USE_BF16 else np.float32

# Problem constants (hardcoded per contract).
B, S, D = 1, 2048, 1024
H, H_KV, HD = 16, 4, 64
W = int(math.ceil(math.sqrt(S)))  # 46
NCORES = 8
SQ = S // NCORES  # 256 queries per core
SKV = 384  # local key span: [qs-128, qs+256)
SIDX = np.arange(W - 1, S, W)  # strided key positions
NS = len(SIDX)  # 44
KT = D // 128  # 8 contraction k-tiles
# jt1 scores psum: heads packed at these free offsets so no matmul output
# crosses a 512-f32 PSUM bank boundary ([0:192],[192:384] | [512:704],[704:896]).
J1OFF = (0, 192, 512, 704)
# i-window (in local query coords) with valid local-attention pairs per j-tile.
WIN = ((0, 64), (0, 192), (128, 256))


def build_nc():
    nc = bacc.Bacc("TRN2", target_bir_lowering=False, debug=False, num_devices=1)
    xkv = nc.dram_tensor("xkv", [D, SKV], MM_DT, kind="ExternalInput").ap()
    xs = nc.dram_tensor("xs", [D, NS], MM_DT, kind="ExternalInput").ap()
    wq = nc.dram_tensor("wq", [D, D], MM_DT, kind="ExternalInput").ap()
    wk = nc.dram_tensor("wk", [D, 256], MM_DT, kind="ExternalInput").ap()
    wv = nc.dram_tensor("wv", [D, 260], MM_DT, kind="ExternalInput").ap()
    wo = nc.dram_tensor("wo", [D, D], MM_DT, kind="ExternalInput").ap()
    m0 = nc.dram_tensor("m0", [128, 256], MM_DT, kind="ExternalInput").ap()
    m1 = nc.dram_tensor("m1", [128, 768], MM_DT, kind="ExternalInput").ap()
    m2 = nc.dram_tensor("m2", [128, 512], MM_DT, kind="ExternalInput").ap()
    ms = nc.dram_tensor("ms", [NS, 1024], MM_DT, kind="ExternalInput").ap()
    ones = nc.dram_tensor("ones64", [1, 64], MM_DT, kind="ExternalInput").ap()
    vones = nc.dram_tensor("vones", [128, 4], MM_DT, kind="ExternalInput").ap()
    y = nc.dram_tensor("y", [SQ, D], F32, kind="ExternalOutput").ap()

    xkv_r = xkv.rearrange("(kt p) s -> kt p s", p=128)
    xs_r = xs.rearrange("(kt p) s -> kt p s", p=128)
    wq_r = wq.rearrange("(kt p) o -> kt p o", p=128)
    wk_r = wk.rearrange("(kt p) o -> kt p o", p=128)
    wv_r = wv.rearrange("(kt p) o -> kt p o", p=128)
    wo_r = wo.rearrange("(kt p) o -> kt p o", p=128)

    with tile.TileContext(nc) as tc:
        with (
            tc.tile_pool(name="consts", bufs=1) as consts,
            tc.tile_pool(name="work", bufs=1) as work,
        ):
            # ---- resident SBUF tensors ----
            xkv_sb = consts.tile([128, KT, SKV], MM_DT)
            xs_sb = consts.tile([128, KT, NS], MM_DT)
            wk_sb = consts.tile([128, KT, 256], MM_DT)
            wv_sb = consts.tile([128, KT, 260], MM_DT)
            wq_sb = consts.tile([128, KT, D], MM_DT)
            wo_sb = consts.tile([128, KT, D], MM_DT)
            m0_sb = consts.tile([128, 256], MM_DT)
            m1_sb = consts.tile([128, 768], MM_DT)
            m2_sb = consts.tile([128, 512], MM_DT)
            ms_sb = consts.tile([NS, 1024], MM_DT)
            for kt in range(KT):
                nc.sync.dma_start(out=xkv_sb[:, kt, :], in_=xkv_r[kt])
                nc.sync.dma_start(out=xs_sb[:, kt, :], in_=xs_r[kt])
                nc.sync.dma_start(out=wk_sb[:, kt, :], in_=wk_r[kt])
                nc.sync.dma_start(out=wv_sb[:, kt, :], in_=wv_r[kt])
            ones_sb = consts.tile([1, 64], MM_DT)
            nc.sync.dma_start(out=ones_sb[:], in_=ones)
            nc.sync.dma_start(out=m0_sb[:], in_=m0)
            nc.sync.dma_start(out=m1_sb[:], in_=m1)
            nc.sync.dma_start(out=m2_sb[:], in_=m2)
            nc.sync.dma_start(out=ms_sb[:], in_=ms)
            for kt in range(KT):
                nc.sync.dma_start(out=wq_sb[:, kt, :], in_=wq_r[kt])
            for kt in range(KT):
                nc.sync.dma_start(out=wo_sb[:, kt, :], in_=wo_r[kt])

            q_sb = work.tile([64, H, SQ], MM_DT)  # q_T per head (d on partitions)
            k_sb = work.tile([64, 4, SKV], MM_DT)  # k_T per kv head
            ks_sb = work.tile([64, 4, NS], MM_DT)
            v_sb = work.tile([128, 3, 260], MM_DT)  # v rows, 65-stride heads
            vs_sb = work.tile([NS, 260], MM_DT)
            attn_sb = work.tile([128, 8, SQ], MM_DT)  # normalized attn_T

            # ---- phase A: projections ----
            with tc.tile_pool(name="ps_proj", bufs=3, space="PSUM") as psp:
                for ot in range(2):  # k_T / ks_T (256 kv channels)
                    kp = psp.tile([128, SKV], F32, tag="proj")
                    for kt in range(KT):
                        nc.tensor.matmul(
                            kp[:],
                            wk_sb[:, kt, bass.ts(ot, 128)],
                            xkv_sb[:, kt, :],
                            start=kt == 0,
                            stop=kt == KT - 1,
                        )
                    nc.any.tensor_copy(k_sb[:, 2 * ot, :], kp[0:64, :])
                    nc.any.tensor_copy(k_sb[:, 2 * ot + 1, :], kp[64:128, :])
                    ksp = psp.tile([128, NS], F32, tag="proj")
                    for kt in range(KT):
                        nc.tensor.matmul(
                            ksp[:],
                            wk_sb[:, kt, bass.ts(ot, 128)],
                            xs_sb[:, kt, :],
                            start=kt == 0,
                            stop=kt == KT - 1,
                        )
                    nc.any.tensor_copy(ks_sb[:, 2 * ot, :], ksp[0:64, :])
                    nc.any.tensor_copy(ks_sb[:, 2 * ot + 1, :], ksp[64:128, :])
                for mt in range(3):  # v rows
                    vp = psp.tile([128, 260], F32, tag="proj")
                    for kt in range(KT):
                        nc.tensor.matmul(
                            vp[:],
                            xkv_sb[:, kt, bass.ts(mt, 128)],
                            wv_sb[:, kt, :],
                            start=kt == 0,
                            stop=kt == KT - 1,
                        )
                    nc.any.tensor_copy(v_sb[:, mt, :], vp[:])
                    ones_cols = v_sb[:, mt, :].rearrange("p (g c) -> p g c", g=4)[
                        :, :, 64
                    ]
                    nc.sync.dma_start(out=ones_cols, in_=vones)
                vsp = psp.tile([NS, 260], F32, tag="proj")
                for kt in range(KT):
                    nc.tensor.matmul(
                        vsp[:],
                        xs_sb[:, kt, :],
                        wv_sb[:, kt, :],
                        start=kt == 0,
                        stop=kt == KT - 1,
                    )
                nc.any.tensor_copy(vs_sb[:], vsp[:])
                vs_ones = vs_sb[:].rearrange("p (g c) -> p g c", g=4)[:, :, 64]
                nc.sync.dma_start(out=vs_ones, in_=vones[0:NS, :])
                for ot in range(8):  # q_T
                    qp = psp.tile([128, SQ], F32, tag="proj")
                    for kt in range(KT):
                        nc.tensor.matmul(
                            qp[:],
                            wq_sb[:, kt, bass.ts(ot, 128)],
                            xkv_sb[:, kt, 128:384],
                            start=kt == 0,
                            stop=kt == KT - 1,
                        )
                    nc.any.tensor_copy(q_sb[:, 2 * ot, :], qp[0:64, :])
                    nc.any.tensor_copy(q_sb[:, 2 * ot + 1, :], qp[64:128, :])

            # ---- phase B: sparse attention per kv-head g ----
            def qh(h, c0, c1):  # q_T slice of head h, query cols [c0:c1)
                return q_sb[:, h, c0:c1]

            with (
                tc.tile_pool(name="ps_s", bufs=2, space="PSUM") as pss,
                tc.tile_pool(name="ps_sw", bufs=1, space="PSUM") as pssw,
                tc.tile_pool(name="ps_pv", bufs=4, space="PSUM") as pspv,
                tc.tile_pool(name="ptiles", bufs=2) as pt,
                tc.tile_pool(name="small", bufs=8) as sm,
            ):
                for g in range(4):
                    # local j-tiles 0/2 (1 psum bank each, shared tag)
                    p_loc = []
                    for jt in (0, 2):
                        w0, w1 = WIN[jt]
                        win = w1 - w0
                        sp = pss.tile([128, 4 * win], F32, tag="sA")
                        for hh in range(4):
                            nc.tensor.matmul(
                                sp[:, hh * win : (hh + 1) * win],
                                k_sb[:, g, bass.ts(jt, 128)],
                                qh(4 * g + hh, w0, w1),
                                start=True,
                                stop=True,
                            )
                        p = pt.tile([128, 4 * win], MM_DT, tag=f"p{jt}")
                        nc.scalar.activation(p[:], sp[:], EXP, scale=0.125)
                        msk = m0_sb if jt == 0 else m2_sb
                        nc.vector.tensor_mul(p[:], p[:], msk[:])
                        p_loc.append(p)
                    p0, p2 = p_loc
                    # local j-tile 1 (2 banks, gapped head layout per J1OFF)
                    s1 = pssw.tile([128, 1024], F32, tag="sB")
                    for hh in range(4):
                        nc.tensor.matmul(
                            s1[:, J1OFF[hh] : J1OFF[hh] + 192],
                            k_sb[:, g, 128:256],
                            qh(4 * g + hh, 0, 192),
                            start=True,
                            stop=True,
                        )
                    p1 = pt.tile([128, 1024], MM_DT, tag="p1")
                    nc.scalar.activation(p1[:, 0:384], s1[:, 0:384], EXP, scale=0.125)
                    nc.scalar.activation(
                        p1[:, 512:896], s1[:, 512:896], EXP, scale=0.125
                    )
                    nc.vector.tensor_mul(p1[:, 0:384], p1[:, 0:384], m1_sb[:, 0:384])
                    nc.vector.tensor_mul(
                        p1[:, 512:896], p1[:, 512:896], m1_sb[:, 384:768]
                    )
                    # strided keys (2 banks, reuses sB tag)
                    ss = pssw.tile([NS, 1024], F32, tag="sB")
                    for hh in range(4):
                        nc.tensor.matmul(
                            ss[:, hh * 256 : (hh + 1) * 256],
                            ks_sb[:, g, :],
                            qh(4 * g + hh, 0, 256),
                            start=True,
                            stop=True,
                        )
                    pstr = pt.tile([NS, 1024], MM_DT, tag="pstr")
                    nc.scalar.activation(pstr[:], ss[:], EXP, scale=0.125)
                    nc.vector.tensor_mul(pstr[:], pstr[:], ms_sb[:])

                    # PV + denominator (ones column) per head
                    vg = 65 * g
                    for hh in range(4):
                        h = 4 * g + hh
                        pv = pspv.tile([65, SQ], F32, tag="pv")
                        nc.tensor.matmul(
                            pv[:],
                            vs_sb[:, vg : vg + 65],
                            pstr[:, hh * 256 : (hh + 1) * 256],
                            start=True,
                            stop=False,
                        )
                        nc.tensor.matmul(
                            pv[:, 0:192],
                            v_sb[:, 1, vg : vg + 65],
                            p1[:, J1OFF[hh] : J1OFF[hh] + 192],
                            start=False,
                            stop=False,
                        )
                        nc.tensor.matmul(
                            pv[:, 0:64],
                            v_sb[:, 0, vg : vg + 65],
                            p0[:, hh * 64 : (hh + 1) * 64],
                            start=False,
                            stop=False,
                        )
                        nc.tensor.matmul(
                            pv[:, 128:256],
                            v_sb[:, 2, vg : vg + 65],
                            p2[:, hh * 128 : (hh + 1) * 128],
                            start=False,
                            stop=True,
                        )
                        rt = sm.tile([1, SQ], MM_DT, tag="recip")
                        with nc.allow_low_precision(reason="f32r recip for matmul"):
                            nc.vector.reciprocal(rt[:], pv[64:65, :])
                        rep_ps = pspv.tile([64, SQ], F32, tag="pv")
                        nc.tensor.matmul(
                            rep_ps[:], ones_sb[:], rt[:], start=True, stop=True
                        )
                        rep = sm.tile([64, SQ], F32, tag="rep")
                        nc.any.tensor_copy(rep[:], rep_ps[:])
                        nc.vector.tensor_mul(
                            attn_sb[64 * (h % 2) : 64 * (h % 2) + 64, h // 2, :],
                            pv[0:64, :],
                            rep[:],
                        )

            # ---- phase C: output projection ----
            with (
                tc.tile_pool(name="ps_y", bufs=2, space="PSUM") as psy,
                tc.tile_pool(name="yout", bufs=2) as yo,
            ):
                # y rows: stationary = attn k-tile (reused for 2 N-chunks),
                # moving = wo 512-wide chunks; output [queries, model-dim].
                for st in range(2):
                    for ch in range(2):
                        yp = psy.tile([128, 512], F32, tag="y")
                        for kt in range(KT):
                            nc.tensor.matmul(
                                yp[:],
                                attn_sb[:, kt, bass.ts(st, 128)],
                                wo_sb[:, kt, bass.ts(ch, 512)],
                                start=kt == 0,
                                stop=kt == KT - 1,
                            )
                        ys = yo.tile([128, 512], F32, tag="ysb")
                        nc.any.tensor_copy(ys[:], yp[:])
                        nc.sync.dma_start(
                            out=y[bass.ts(st, 128), bass.ts(ch, 512)], in_=ys[:]
                        )
    nc.compile()
    return nc


def host_prep(x, Wq, Wk, Wv, Wo):
    """Build per-core input maps (pure data reordering, no FLOPs)."""
    x2 = np.asarray(x, np.float32).reshape(S, D)
    xT = np.ascontiguousarray(x2.T)  # [D, S]
    xpad = np.zeros((D, 128 + S), np.float32)
    xpad[:, 128:] = xT
    xs = np.ascontiguousarray(xT[:, SIDX])  # [D, 44]
    wq = np.ascontiguousarray(np.asarray(Wq, np.float32).T)
    wk = np.ascontiguousarray(np.asarray(Wk, np.float32).T)
    wvT = np.asarray(Wv, np.float32).T  # [D, 256]
    wv = np.zeros((D, 260), np.float32)
    for g in range(4):
        wv[:, 65 * g : 65 * g + 64] = wvT[:, 64 * g : 64 * g + 64]
    wo = np.ascontiguousarray(np.asarray(Wo, np.float32).T)

    in_maps = []
    for c in range(NCORES):
        qs = SQ * c
        xkv = np.ascontiguousarray(xpad[:, qs : qs + SKV])
        ig = qs + np.arange(SQ)  # global query index per local col
        jg = qs - 128 + np.arange(SKV)  # global key index per local j row
        diff = ig[None, :] - jg[:, None]  # [384, 256]
        loc_valid = (diff >= 0) & (diff <= W - 1) & (jg[:, None] >= 0)
        masks = []
        for jt in range(3):
            w0, w1 = WIN[jt]
            base = loc_valid[128 * jt : 128 * (jt + 1), w0:w1].astype(np.float32)
            masks.append(np.ascontiguousarray(np.tile(base, (1, 4))))
        str_base = (SIDX[:, None] <= ig[None, :] - W).astype(np.float32)  # [44,256]
        msk_str = np.ascontiguousarray(np.tile(str_base, (1, 4)))
        in_maps.append(
            {
                "xkv": xkv.astype(NP_DT),
                "xs": xs.astype(NP_DT),
                "wq": wq.astype(NP_DT),
                "wk": wk.astype(NP_DT),
                "wv": wv.astype(NP_DT),
                "wo": wo.astype(NP_DT),
                "m0": masks[0].astype(NP_DT),
                "m1": masks[1].astype(NP_DT),
                "m2": masks[2].astype(NP_DT),
                "ms": msk_str.astype(NP_DT),
                "ones64": np.ones((1, 64), NP_DT),
                "vones": np.ones((128, 4), NP_DT),
            }
        )
    return in_maps


_NC_CACHE = {}


def get_nc():
    if "nc" not in _NC_CACHE:
        _NC_CACHE["nc"] = build_nc()
    return _NC_CACHE["nc"]


def kernel(x, Wq, Wk, Wv, Wo):
    nc = get_nc()
    in_maps = host_prep(x, Wq, Wk, Wv, Wo)
    res = run_bass_kernel_spmd(nc, in_maps, core_ids=list(range(NCORES)))
    yrows = np.concatenate([r["y"] for r in res.results], axis=0)  # [S, D]
    return np.ascontiguousarray(yrows).reshape(B, S, D).astype(np.float32)

